# revision 12
# baseline (speedup 1.0000x reference)
"""Trainium2 Bass kernel for nn_Network_17506286698721 (GAT message passing).

Strategy:
  - The graph (edges, per-node static features) is identical across the
    B*T = 128 ticks; only the per-channel signal x[0,0,:,t] varies.
  - Algebraic reduction: with h = nodes @ W_gat + b_gat,
      s[n,t]   = cs0*sig[cA[n],t] + cs5*sig[cB[n],t] + tau_s*t + const_s[n]
      d[n,t]   = cd0*sig[cA[n],t] + cd5*sig[cB[n],t] + tau_d*t + const_d[n]
      hmix[n,t] (= h @ W_mlp)
               = al*sig[cA[n],t] + be*sig[cB[n],t] + ga*t + chmix[n]
    and since W_mlp applies linearly after aggregation, the output is
      out[t,n] = sigmoid( N0/D0 + ga*t + b_mlp ),
      D0 = sum_e exp(leaky(s[src]+d[n])), N0 = sum_e exp(..)*(hmix[src]-ga*t)
    e_att is bounded (<~2) so unshifted exp is safe in f32, and the
    reference's +1e-9 epsilon is negligible (denominators >= exp(max)).
  - Sharding: nodes (dst) are dealt round-robin per degree bucket across
    the 8 cores; each core processes all 128 ticks for its nodes.
  - Layout: node-major degree buckets [node_partition(128), K slots, 128
    ticks]. Segment sums = strided tensor_reduce over K. dst-side values
    are per-partition rows; per-slot consts are per-partition scalars.
  - Gathers of sig rows (512B each) via gpsimd.dma_gather.
"""
import sys, types
import numpy as np
from contextlib import ExitStack

NCH, T, N, E = 1536, 128, 30000, 120000
N_CORES = 8
DEG_MERGE = {9: 10, 11: 12, 13: 15, 14: 15}  # deg -> padded K (else exact)


def _install_ntff_hook():
    try:
        import antenv.axon_hooks  # noqa
    except ImportError:
        try:
            from trn_agent_boot.trn_boot import _ntff_profile_via_ctypes
            _mod = types.ModuleType('antenv.axon_hooks')
            _mod.get_axon_ntff_profile_hook = (
                lambda: _ntff_profile_via_ctypes('/opt/axon/libaxon_pjrt.so'))
            _mod.set_axon_ntff_profile_hook = lambda h: None
            sys.modules['antenv.axon_hooks'] = _mod
        except Exception:
            pass


def _wrap_idx(idx):
    """dma_gather index layout: [128, n//16] int16, idx i at [i%16 (+16g), i//16]."""
    n = len(idx)
    assert n % 16 == 0
    w = np.zeros((128, n // 16), np.int16)
    blk = idx.reshape(n // 16, 16).T.astype(np.int16)
    for g in range(8):
        w[g * 16:(g + 1) * 16, :] = blk
    return w


def _prep(x, chan_p0, chan_p1, chan_p2, gidx_01, gidx_12, gidx_20,
          ray_01, ray_12, ray_20, edges, W_gat, a_src, a_dst, b_gat,
          W_mlp, b_mlp):
    """Host-side index/constant preprocessing. Returns per-core tensors +
    assembly info."""
    W = W_gat.astype(np.float64)
    # per-crossing-node channel ids + static features
    cA = np.concatenate([chan_p0[gidx_01[:, 0]], chan_p1[gidx_12[:, 0]],
                         chan_p2[gidx_20[:, 0]]]).astype(np.int64)
    cB = np.concatenate([chan_p1[gidx_01[:, 1]], chan_p2[gidx_12[:, 1]],
                         chan_p0[gidx_20[:, 1]]]).astype(np.int64)
    wA = np.concatenate([gidx_01[:, 0], gidx_12[:, 0], gidx_20[:, 0]]).astype(np.float64)
    wB = np.concatenate([gidx_01[:, 1], gidx_12[:, 1], gidx_20[:, 1]]).astype(np.float64)
    pA = np.concatenate([np.full(10000, 0.), np.full(10000, 1.), np.full(10000, 2.)])
    pB = np.concatenate([np.full(10000, 1.), np.full(10000, 2.), np.full(10000, 0.)])
    ray = np.concatenate([ray_01, ray_12, ray_20]).astype(np.float64)

    # const_h_node[n, c] over static dims (1,2,3,4, 6,7,8,9, 10,11) + b_gat
    statf = np.stack([wA, cA.astype(np.float64), np.zeros(N), pA,
                      wB, cB.astype(np.float64), np.zeros(N), pB,
                      ray[:, 0], ray[:, 1]], axis=1)            # [N,10]
    Wst = W[[1, 2, 3, 4, 6, 7, 8, 9, 10, 11], :]                 # [10,4]
    const_h = statf @ Wst + b_gat.astype(np.float64)             # [N,4]
    a_s, a_d, wm = (a_src.astype(np.float64), a_dst.astype(np.float64),
                    W_mlp[:, 0].astype(np.float64))
    const_s = (const_h @ a_s).astype(np.float32)
    const_d = (const_h @ a_d).astype(np.float32)
    chmix = (const_h @ wm).astype(np.float32)
    sc = dict(cs0=float(W[0] @ a_s), cs5=float(W[5] @ a_s),
              cd0=float(W[0] @ a_d), cd5=float(W[5] @ a_d),
              tau=float(W[12] @ a_s + W[12] @ a_d),
              al=float(W[0] @ wm), be=float(W[5] @ wm),
              ga=float(W[12] @ wm), bm=float(b_mlp[0]))

    # group edges by dst
    src, dst = edges[0].astype(np.int64), edges[1].astype(np.int64)
    order = np.argsort(dst, kind='stable')
    src_s, dst_s = src[order], dst[order]
    deg = np.bincount(dst, minlength=N)
    starts = np.zeros(N + 1, np.int64)
    np.cumsum(deg, out=starts[1:])

    # degree buckets (merged) and round-robin core deal
    Kof = np.array([DEG_MERGE.get(k, k) for k in range(int(deg.max()) + 1)])
    nodeK = Kof[deg]
    kvals = sorted(set(nodeK[deg > 0].tolist()))
    # per (bucket, core): node lists
    percore = [{k: [] for k in kvals} for _ in range(N_CORES)]
    for k in kvals:
        nodes_k = np.flatnonzero(nodeK == k)
        for i, n in enumerate(nodes_k):
            percore[i % N_CORES][k].append(n)
    ntil = {k: max((len(percore[c][k]) + 127) // 128 for c in range(N_CORES))
            for k in kvals}

    # build per-core arrays with identical shapes (SPMD)
    NTILS = sum(ntil.values())
    NBLKS = sum(ntil[k] * k for k in kvals)
    cores = []
    node_map = np.full((N_CORES, NTILS * 128), -1, np.int64)  # -> node id
    for c in range(N_CORES):
        nd_idx = np.zeros(NTILS * 2 * 128, np.int64)     # [tile]{A,B} blocks
        # slot blocks per tile: [A_k0..A_k{K-1}, B_k0..B_k{K-1}] (contiguous A/B)
        sl_idx = np.zeros(NBLKS * 2 * 128, np.int64)
        CD = np.zeros((128, NTILS), np.float32)
        CS = np.zeros((128, NBLKS), np.float32)
        CH = np.zeros((128, NBLKS), np.float32)
        toff, boff = 0, 0
        for k in kvals:
            nl = percore[c][k]
            for t in range(ntil[k]):
                tn = nl[t * 128:(t + 1) * 128]
                tidx = toff + t
                sb = 2 * boff + t * 2 * k    # slot-gather block base for tile
                nb = 2 * toff  # bucket's nd block base: A tiles then B tiles
                for p, n in enumerate(tn):
                    node_map[c, tidx * 128 + p] = n
                    nd_idx[(nb + t) * 128 + p] = cA[n]
                    nd_idx[(nb + ntil[k] + t) * 128 + p] = cB[n]
                    CD[p, tidx] = const_d[n]
                    es = src_s[starts[n]:starts[n] + deg[n]]
                    for j in range(k):
                        blk = boff + t * k + j
                        if j < deg[n]:
                            s = es[j]
                            sl_idx[(sb + j) * 128 + p] = cA[s]
                            sl_idx[(sb + k + j) * 128 + p] = cB[s]
                            CS[p, blk] = const_s[s]
                            CH[p, blk] = chmix[s]
                        else:
                            CS[p, blk] = -1e30     # pad slot -> w = 0
            toff += ntil[k]
            boff += ntil[k] * k
        cores.append(dict(nd_idx=_wrap_idx(nd_idx), sl_idx=_wrap_idx(sl_idx),
                          CD=CD, CS=CS, CH=CH))
    buckets = [(k, ntil[k]) for k in kvals]
    return cores, buckets, NTILS, NBLKS, node_map, sc, deg


_KERNEL_CACHE = {}


def _build(buckets, NTILS, NBLKS, sc):
    import concourse.bass as bass
    import concourse.tile as tile
    from concourse import bacc, mybir
    dt = mybir.dt
    key = (tuple(buckets), NTILS, NBLKS, tuple(sorted(sc.items())))
    if key in _KERNEL_CACHE:
        return _KERNEL_CACHE[key]

    nc = bacc.Bacc("TRN2", target_bir_lowering=False, debug=False,
                   enable_asserts=False, num_devices=N_CORES)
    sig_d = nc.dram_tensor("sig", [NCH, T], dt.float32, kind="ExternalInput").ap()
    ndi_d = nc.dram_tensor("nd_idx", [128, NTILS * 2 * 8], dt.int16,
                           kind="ExternalInput").ap()
    sli_d = nc.dram_tensor("sl_idx", [128, NBLKS * 2 * 8], dt.int16,
                           kind="ExternalInput").ap()
    CD_d = nc.dram_tensor("CD", [128, NTILS], dt.float32, kind="ExternalInput").ap()
    CS_d = nc.dram_tensor("CS", [128, NBLKS], dt.float32, kind="ExternalInput").ap()
    CH_d = nc.dram_tensor("CH", [128, NBLKS], dt.float32, kind="ExternalInput").ap()
    out_d = nc.dram_tensor("out", [128, NTILS * T], dt.float32,
                           kind="ExternalOutput").ap()

    with tile.TileContext(nc) as tc, ExitStack() as ctx:
        cpool = ctx.enter_context(tc.tile_pool(name="consts", bufs=1))
        gpool = ctx.enter_context(tc.tile_pool(name="gath", bufs=2))
        wpool = ctx.enter_context(tc.tile_pool(name="work", bufs=2))
        opool = ctx.enter_context(tc.tile_pool(name="outp", bufs=1))

        ndi = cpool.tile([128, NTILS * 2 * 8], dt.int16)
        nc.sync.dma_start(ndi[:], ndi_d[:])
        sli = cpool.tile([128, NBLKS * 2 * 8], dt.int16)
        nc.sync.dma_start(sli[:], sli_d[:])
        CD = cpool.tile([128, NTILS], dt.float32)
        nc.sync.dma_start(CD[:], CD_d[:])
        CS = cpool.tile([128, NBLKS], dt.float32)
        nc.sync.dma_start(CS[:], CS_d[:])
        CH = cpool.tile([128, NBLKS], dt.float32)
        nc.sync.dma_start(CH[:], CH_d[:])

        # tick ramp [128, 128] (value = free index) as f32
        ioti = cpool.tile([128, T], dt.int32)
        nc.gpsimd.iota(ioti[:], pattern=[[1, T]], base=0, channel_multiplier=0)
        TB = cpool.tile([128, T], dt.float32)
        nc.vector.tensor_copy(TB[:], ioti[:])
        bias_t = cpool.tile([128, 1], dt.float32)
        nc.vector.memset(bias_t[:], sc["bm"])

        AL = mybir.AluOpType
        stage = opool.tile([128, NTILS * T], dt.float32)
        toff, boff = 0, 0
        for (K, ntl) in buckets:
            # per-bucket gathers: node rows (A tiles then B tiles) + slot rows
            Gn = gpool.tile([128, 2 * ntl, T], dt.float32, tag="gn")
            nc.gpsimd.dma_gather(
                Gn[:], sig_d[:], ndi[:, (toff * 2) * 8:((toff + ntl) * 2) * 8],
                num_idxs=2 * ntl * 128, num_idxs_reg=2 * ntl * 128,
                elem_size=T, single_packet=False)
            Gs = gpool.tile([128, 2 * K * ntl, T], dt.float32, tag="gs")
            nc.gpsimd.dma_gather(
                Gs[:], sig_d[:], sli[:, (2 * boff) * 8:(2 * (boff + ntl * K)) * 8],
                num_idxs=2 * K * ntl * 128, num_idxs_reg=2 * K * ntl * 128,
                elem_size=T, single_packet=False)
            # d rows for all tiles: cd0*sigA + cd5*sigB + const_d + tau*t
            dn = wpool.tile([128, ntl, T], dt.float32, tag="dn")
            CDb = CD[:, toff:toff + ntl].rearrange(
                "p (n a) -> p n a", a=1).broadcast_to([128, ntl, T])
            TBb = TB[:].rearrange("p (a t) -> p a t", a=1).broadcast_to([128, ntl, T])
            nc.vector.scalar_tensor_tensor(dn[:], TBb, sc["tau"], CDb,
                                           op0=AL.mult, op1=AL.add)
            nc.vector.scalar_tensor_tensor(dn[:], Gn[:, 0:ntl, :], sc["cd0"],
                                           dn[:], op0=AL.mult, op1=AL.add)
            nc.vector.scalar_tensor_tensor(dn[:], Gn[:, ntl:2 * ntl, :], sc["cd5"],
                                           dn[:], op0=AL.mult, op1=AL.add)

            # CS/CH widened along ticks (ACT copies, 3D broadcast)
            CSw = wpool.tile([128, ntl * K, T], dt.float32, tag="csw")
            csb = CS[:, boff:boff + ntl * K].rearrange(
                "p (b a) -> p b a", a=1).broadcast_to([128, ntl * K, T])
            nc.scalar.copy(CSw[:], csb)
            CHw = wpool.tile([128, ntl * K, T], dt.float32, tag="csw")
            chb = CH[:, boff:boff + ntl * K].rearrange(
                "p (b a) -> p b a", a=1).broadcast_to([128, ntl * K, T])
            nc.scalar.copy(CHw[:], chb)

            t1 = wpool.tile([128, ntl * K, T], dt.float32, tag="t1")
            h = wpool.tile([128, ntl * K, T], dt.float32, tag="h")
            for t in range(ntl):
                A3 = Gs[:, t * 2 * K:t * 2 * K + K, :]
                B3 = Gs[:, t * 2 * K + K:t * 2 * K + 2 * K, :]
                dnb = dn[:, t, :].rearrange("p (a t) -> p a t", a=1).broadcast_to(
                    [128, K, T])
                t1s = t1[:, t * K:(t + 1) * K, :]
                nc.vector.scalar_tensor_tensor(t1s, A3, sc["cs0"], dnb,
                                               op0=AL.mult, op1=AL.add)
                nc.vector.scalar_tensor_tensor(t1s, B3, sc["cs5"], t1s,
                                               op0=AL.mult, op1=AL.add)
                hs = h[:, t * K:(t + 1) * K, :]
                nc.vector.scalar_tensor_tensor(hs, A3, sc["al"],
                                               CHw[:, t * K:(t + 1) * K, :],
                                               op0=AL.mult, op1=AL.add)
                nc.vector.scalar_tensor_tensor(hs, B3, sc["be"], hs,
                                               op0=AL.mult, op1=AL.add)
            # pre = t1 + CS; leaky; exp; h*w  (bucket-wide)
            nc.vector.tensor_add(t1[:], t1[:], CSw[:])
            nc.vector.scalar_tensor_tensor(t1[:], t1[:], 0.2, t1[:],
                                           op0=AL.mult, op1=AL.max)
            nc.scalar.activation(t1[:], t1[:], mybir.ActivationFunctionType.Exp)
            nc.vector.tensor_mul(h[:], h[:], t1[:])
            # reduces over K (strided innermost, per tile)
            D0 = wpool.tile([128, ntl, T], dt.float32, tag="d0")
            N0 = wpool.tile([128, ntl, T], dt.float32, tag="n0")
            for t in range(ntl):
                nc.vector.tensor_reduce(
                    D0[:, t, :], t1[:, t * K:(t + 1) * K, :].rearrange(
                        "p k t -> p t k"),
                    axis=mybir.AxisListType.X, op=AL.add)
                nc.vector.tensor_reduce(
                    N0[:, t, :], h[:, t * K:(t + 1) * K, :].rearrange(
                        "p k t -> p t k"),
                    axis=mybir.AxisListType.X, op=AL.add)
            # g = N0/D0 + ga*t  (sigmoid batched at the end)
            rc = wpool.tile([128, ntl, T], dt.float32, tag="rc")
            nc.vector.reciprocal(rc[:], D0[:])
            nc.vector.tensor_mul(rc[:], rc[:], N0[:])
            st = stage[:, toff * T:(toff + ntl) * T].rearrange(
                "p (n t) -> p n t", n=ntl)
            nc.vector.scalar_tensor_tensor(st, TBb, sc["ga"], rc[:],
                                           op0=AL.mult, op1=AL.add)
            toff += ntl
            boff += ntl * K
        nc.scalar.activation(stage[:], stage[:],
                             mybir.ActivationFunctionType.Sigmoid,
                             bias=bias_t[:], scale=1.0)
        nc.sync.dma_start(out_d[:], stage[:])
    nc.compile()
    _KERNEL_CACHE[key] = nc
    return nc


def kernel(**inputs):
    _install_ntff_hook()
    np_in = {k: np.asarray(v) for k, v in inputs.items()}
    cores, buckets, NTILS, NBLKS, node_map, sc, deg = _prep(**np_in)
    nc = _build(buckets, NTILS, NBLKS, sc)

    sig = np.ascontiguousarray(np_in["x"][0, 0]).astype(np.float32)  # [1536,128]
    in_maps = []
    for c in range(N_CORES):
        d = cores[c]
        in_maps.append(dict(sig=sig, nd_idx=d["nd_idx"], sl_idx=d["sl_idx"],
                            CD=d["CD"], CS=d["CS"], CH=d["CH"]))
    import os
    from concourse.bass_utils import run_bass_kernel_spmd
    trace = bool(int(os.environ.get("GAT_KERNEL_TRACE", "0")))
    res = run_bass_kernel_spmd(nc, in_maps, core_ids=list(range(N_CORES)),
                               trace=trace)
    kernel.last_exec_time_ns = res.exec_time_ns

    # assemble: out[t, node]
    out = np.empty((1, T, N, 1), np.float32)
    # empty nodes: sigmoid(b_mlp)
    out[:] = 1.0 / (1.0 + np.exp(-np.float32(sc["bm"])))
    for c in range(N_CORES):
        oc = res.results[c]["out"].reshape(128, NTILS, T)
        pos = np.flatnonzero(node_map[c] >= 0)
        nodes = node_map[c][pos]
        out[0, :, nodes, 0] = oc[pos % 128, pos // 128, :]
    return out
